# revision 2
# baseline (speedup 1.0000x reference)
"""TRN2 Bass kernel for nn_Network: 3-layer MLP (256-512-512-256) with custom
per-node activation a0*tanh(x)*sin(a1*x+a2) + a3*x + a4, followed by softmax.

Strategy (8 NeuronCores, data-parallel over batch):
- Host pre-transposes data to feature-major [256, B] and slices per core.
- Feature-major dataflow on device: activations live as [features, batch]
  tiles; weights (host-folded) are the stationary matmul operand, so each
  layer's output is already feature-major for the next layer.
- Algebraic folds (host, fp64):
    h_l = a0*t*s + a3*x + a4  with  t=tanh(x), s=sin(a1*x+a2)
  is rewritten as g_l = x + r*t*s (r = a0/a3), h_l = a3*g_l + a4, and both
  the diag(a3) scale and the a4/bias constants are folded into the next
  layer's weights/biases:  W'_{l+1} = diag(a3_l) W_{l+1},
  c_{l+1} = W_{l+1}^T a4_l + b_{l+1}.
- fp32 (exact) matmuls on PE; psum evacuated by ACT (tanh, affine-u) and DVE
  (round, residual, fused scalar_tensor_tensor ops).
- sin computed as Sin(2*pi*v) after explicit range reduction:
  u = (a1*x_hat+a2)/2pi  (ACT affine from PSUM), k = round(u) (DVE magic
  const), v = u - k (DVE), s = Sin(v; scale=2pi) -- spline only valid on
  [-pi, pi].
- softmax in feature-major layout using gpsimd partition_all_reduce (max and
  add give broadcast results), exp/ln on ACT, final scale on DVE.
- Host transposes the feature-major output back to [B, 256].
"""
import numpy as np

import concourse.bass as bass
import concourse.bass_isa as bass_isa
import concourse.mybir as mybir
import concourse.tile as tile
from concourse import bacc
from concourse.bass_utils import run_bass_kernel_spmd

F32 = mybir.dt.float32
ACTF = mybir.ActivationFunctionType
ALU = mybir.AluOpType

N_CORES = 8
B = 65536
B_LOC = B // N_CORES           # 8192 rows per core
NB = 1024                       # batch columns per mega-tile
N_MEGA = B_LOC // NB            # 8
MM_N = 512                      # moving-operand width per matmul (fp32 limit)
LAYERS = [256, 512, 512, 256]
C_RND = float(1.5 * 2 ** 23)    # fp32 round-to-int magic constant
TWO_PI = float(2 * np.pi)

_CACHE = {}


def _build_program(repeat=1):
    """Build (and cache) the Bass program. Returns (nc, meta)."""
    key = ("prog", repeat)
    if key in _CACHE:
        return _CACHE[key]

    n_wcols = 0
    wslots = {}  # (layer, ktile, ftile) -> column offset
    for li, (n_in, n_out) in enumerate(zip(LAYERS[:-1], LAYERS[1:]), start=1):
        for k in range(n_in // 128):
            for f in range(n_out // 128):
                wslots[(li, k, f)] = n_wcols
                n_wcols += 128

    # params: per (layer, ftile): [c, sinScale, sinBias, r]; plus layer-3 [a3, a4]
    pslots = {}
    n_pcols = 0
    for li, n_out in ((1, 512), (2, 512), (3, 256)):
        for f in range(n_out // 128):
            pslots[(li, f)] = n_pcols
            n_pcols += 4
    l3slots = {}
    for f in range(2):
        l3slots[f] = n_pcols
        n_pcols += 2

    nc = bacc.Bacc("TRN2", target_bir_lowering=False, debug=False,
                   num_devices=N_CORES)
    xt_d = nc.dram_tensor("xt", [256, B_LOC], F32, kind="ExternalInput").ap()
    wt_d = nc.dram_tensor("wt", [128, n_wcols], F32, kind="ExternalInput").ap()
    pp_d = nc.dram_tensor("pp", [128, n_pcols], F32, kind="ExternalInput").ap()
    yt_d = nc.dram_tensor("yt", [256, B_LOC], F32, kind="ExternalOutput").ap()

    with tile.TileContext(nc, num_cores=N_CORES) as tc:
        with tc.tile_pool(name="const", bufs=1) as cpool, \
             tc.tile_pool(name="io", bufs=1) as iopool, \
             tc.tile_pool(name="work", bufs=1) as wpool, \
             tc.tile_pool(name="psum", bufs=2, space="PSUM") as pspool:

            wt = cpool.tile([128, n_wcols], F32)
            nc.sync.dma_start(wt[:], wt_d)
            pp = cpool.tile([128, n_pcols], F32)
            nc.sync.dma_start(pp[:], pp_d)

            def wtile(li, k, f):
                o = wslots[(li, k, f)]
                return wt[:, o:o + 128]

            def pcol(li, f, idx):
                o = pslots[(li, f)] + idx
                return pp[:, o:o + 1]

            def l3col(f, idx):
                o = l3slots[f] + idx
                return pp[:, o:o + 1]

            for rep_m in range(N_MEGA * repeat):
                m = rep_m % N_MEGA
                bsl = slice(m * NB, (m + 1) * NB)
                # ---- input tiles (feature-major) ----
                g_prev = []
                for k in range(2):
                    xt_t = iopool.tile([128, NB], F32, tag="xt", bufs=4,
                                       name=f"xt_{rep_m}_{k}")
                    nc.sync.dma_start(xt_t[:], xt_d[k * 128:(k + 1) * 128, bsl])
                    g_prev.append(xt_t)

                vtiles = []
                for li, (n_in, n_out) in enumerate(
                        zip(LAYERS[:-1], LAYERS[1:]), start=1):
                    nk, nf = n_in // 128, n_out // 128
                    g_next = []
                    for f in range(nf):
                        ps = pspool.tile([128, NB], F32, tag="x",
                                         name=f"ps_{rep_m}_{li}_{f}")
                        for b in range(NB // MM_N):
                            for k in range(nk):
                                nc.tensor.matmul(
                                    ps[:, b * MM_N:(b + 1) * MM_N],
                                    wtile(li, k, f),
                                    g_prev[k][:, b * MM_N:(b + 1) * MM_N],
                                    start=(k == 0), stop=(k == nk - 1))
                        # ---- activation: g = x + c + r*tanh(x+c)*sin(2pi*v) ----
                        t_t = wpool.tile([128, NB], F32, tag="t", bufs=2,
                                         name=f"t_{rep_m}_{li}_{f}")
                        nc.scalar.activation(t_t[:], ps[:], ACTF.Tanh,
                                             bias=pcol(li, f, 0), scale=1.0)
                        u_t = wpool.tile([128, NB], F32, tag="u", bufs=2,
                                         name=f"u_{rep_m}_{li}_{f}")
                        nc.scalar.activation(u_t[:], ps[:], ACTF.Identity,
                                             bias=pcol(li, f, 2),
                                             scale=pcol(li, f, 1))
                        k_t = wpool.tile([128, NB], F32, tag="k", bufs=2,
                                         name=f"k_{rep_m}_{li}_{f}")
                        nc.vector.tensor_scalar(k_t[:], u_t[:], C_RND, C_RND,
                                                ALU.add, ALU.subtract)
                        v_t = wpool.tile([128, NB], F32, tag="v", bufs=2,
                                         name=f"v_{rep_m}_{li}_{f}")
                        nc.vector.tensor_tensor(v_t[:], u_t[:], k_t[:],
                                                ALU.subtract)
                        s_t = wpool.tile([128, NB], F32, tag="s", bufs=2,
                                         name=f"s_{rep_m}_{li}_{f}")
                        nc.scalar.activation(s_t[:], v_t[:], ACTF.Sin,
                                             bias=0.0, scale=TWO_PI)
                        q_t = wpool.tile([128, NB], F32, tag="q", bufs=2,
                                         name=f"q_{rep_m}_{li}_{f}")
                        nc.vector.scalar_tensor_tensor(
                            q_t[:], t_t[:], pcol(li, f, 3), s_t[:],
                            ALU.mult, ALU.mult)
                        gtag = f"g{li}"
                        g_t = wpool.tile([128, NB], F32, tag=gtag,
                                         bufs=(5 if li < 3 else 3),
                                         name=f"g_{rep_m}_{li}_{f}")
                        nc.vector.scalar_tensor_tensor(
                            g_t[:], ps[:], pcol(li, f, 0), q_t[:],
                            ALU.add, ALU.add)
                        g_next.append(g_t)
                    g_prev = g_next

                # ---- softmax over 256 features (2 g3 tiles), feature-major --
                for f in range(2):
                    # v = a3*g3 + a4  (in place over g3)
                    nc.vector.tensor_scalar(g_prev[f][:], g_prev[f][:],
                                            l3col(f, 0), l3col(f, 1),
                                            ALU.mult, ALU.add)
                    vtiles.append(g_prev[f])
                vm = wpool.tile([128, NB], F32, tag="vm", bufs=2,
                                name=f"vm_{rep_m}")
                nc.vector.tensor_tensor(vm[:], vtiles[0][:], vtiles[1][:],
                                        ALU.max)
                mb = wpool.tile([128, NB], F32, tag="mb", bufs=2,
                                name=f"mb_{rep_m}")
                nc.gpsimd.partition_all_reduce(mb[:], vm[:], channels=128,
                                               reduce_op=bass_isa.ReduceOp.max)
                es = wpool.tile([128, NB], F32, tag="es", bufs=2,
                                name=f"es_{rep_m}")
                for f in range(2):
                    # d = v - mb (in place), e = exp(d) (in place)
                    nc.vector.tensor_tensor(vtiles[f][:], vtiles[f][:], mb[:],
                                            ALU.subtract)
                    nc.scalar.activation(vtiles[f][:], vtiles[f][:], ACTF.Exp)
                nc.vector.tensor_tensor(es[:], vtiles[0][:], vtiles[1][:],
                                        ALU.add)
                sb = wpool.tile([128, NB], F32, tag="sb", bufs=2,
                                name=f"sb_{rep_m}")
                nc.gpsimd.partition_all_reduce(sb[:], es[:], channels=128,
                                               reduce_op=bass_isa.ReduceOp.add)
                # rs = 1/sb via exp(-ln(sb)), in place on sb
                nc.scalar.activation(sb[:], sb[:], ACTF.Ln)
                nc.scalar.activation(sb[:], sb[:], ACTF.Exp, scale=-1.0)
                for f in range(2):
                    nc.vector.tensor_tensor(vtiles[f][:], vtiles[f][:], sb[:],
                                            ALU.mult)
                    nc.sync.dma_start(yt_d[f * 128:(f + 1) * 128, bsl],
                                      vtiles[f][:])

    nc.compile()
    _CACHE[key] = (nc, wslots, pslots, l3slots, n_wcols, n_pcols)
    return _CACHE[key]


def _prep_host(inputs, repeat=1):
    """Fold params on host (fp64) and pack device input tensors."""
    W = [None, inputs["W1"].astype(np.float64), inputs["W2"].astype(np.float64),
         inputs["W3"].astype(np.float64)]
    bvec = [None, inputs["b1"].astype(np.float64), inputs["b2"].astype(np.float64),
            inputs["b3"].astype(np.float64)]
    a = [None, inputs["a1"].astype(np.float64), inputs["a2"].astype(np.float64),
         inputs["a3"].astype(np.float64)]

    a3c = [None] + [np.maximum(a[li][:, 3], 1e-20) for li in (1, 2, 3)]
    r = [None] + [a[li][:, 0] / a3c[li] for li in (1, 2, 3)]

    # folded weights / biases
    Wp = [None, W[1],
          W[2] * a3c[1][:, None],
          W[3] * a3c[2][:, None]]
    c = [None, bvec[1],
         W[2].T @ a[1][:, 4] + bvec[2],
         W[3].T @ a[2][:, 4] + bvec[3]]

    nc, wslots, pslots, l3slots, n_wcols, n_pcols = _build_program(repeat)

    wt = np.zeros((128, n_wcols), np.float32)
    for (li, k, f), o in wslots.items():
        wt[:, o:o + 128] = Wp[li][k * 128:(k + 1) * 128,
                                  f * 128:(f + 1) * 128].astype(np.float32)

    pp = np.zeros((128, n_pcols), np.float32)
    inv2pi = 1.0 / (2 * np.pi)
    for (li, f), o in pslots.items():
        sl = slice(f * 128, (f + 1) * 128)
        pp[:, o + 0] = c[li][sl].astype(np.float32)
        pp[:, o + 1] = (a[li][sl, 1] * inv2pi).astype(np.float32)
        pp[:, o + 2] = ((a[li][sl, 1] * c[li][sl] + a[li][sl, 2]) * inv2pi
                        ).astype(np.float32)
        pp[:, o + 3] = r[li][sl].astype(np.float32)
    for f, o in l3slots.items():
        sl = slice(f * 128, (f + 1) * 128)
        pp[:, o + 0] = a3c[3][sl].astype(np.float32)
        pp[:, o + 1] = a[3][sl, 4].astype(np.float32)

    dataT = np.ascontiguousarray(inputs["data"].astype(np.float32).T)
    in_maps = []
    for i in range(N_CORES):
        xt = np.ascontiguousarray(dataT[:, i * B_LOC:(i + 1) * B_LOC])
        in_maps.append({"xt": xt, "wt": wt, "pp": pp})
    return nc, in_maps


def kernel(**inputs):
    nc, in_maps = _prep_host(inputs)
    res = run_bass_kernel_spmd(nc, in_maps, list(range(N_CORES)))
    out = np.empty((B, LAYERS[-1]), np.float32)
    for i in range(N_CORES):
        out[i * B_LOC:(i + 1) * B_LOC, :] = res.results[i]["yt"].T
    return out


if __name__ == "__main__":
    # smoke test with random data
    rng = np.random.default_rng(0)
    inp = {"data": rng.standard_normal((B, 256), dtype=np.float32)}
    for i, (n_in, n_out) in enumerate(zip(LAYERS[:-1], LAYERS[1:])):
        inp[f"W{i+1}"] = rng.random((n_in, n_out), dtype=np.float32)
        inp[f"b{i+1}"] = np.zeros((n_out,), np.float32)
        inp[f"a{i+1}"] = rng.random((n_out, 5), dtype=np.float32)
    y = kernel(**inp)
    print("out", y.shape, y.dtype, y.min(), y.max())
